# revision 17
# baseline (speedup 1.0000x reference)
"""Multi-head attention (b=2, sq=skv=2048, dim=1024, 16 heads x 64) on 8 TRN2
NeuronCores.

Sharding: 2 heads per core (head-parallel), with the matching tensor-parallel
column slice of W_qkv and row slice of W_out.  Each core computes a partial
output projection over its 128 head-dims; the all-reduce of the 8 partials
(+ bias) happens on the host during unshard.

Per-core kernel (bf16 compute, fp32 PSUM accumulation):
  phase 1: qT/kT/vT = W.T @ x.T ([128 = 2 heads x 64 dims, tokens]); v is
           additionally PE-transposed to natural [token, dim] layout with a
           ones column appended (denominator trick).
  phase 2: per (batch, q-tile, k-tile): scoresT for both heads ([k-tokens, q])
           in one 2-bank PSUM group; one exp ACTIVATE over the group (scale
           1/8 fused, no max subtraction -- scores range +-10); PV matmuls
           accumulate [v | 1].T @ expT over the 16 k-tiles giving unnormalized
           outT plus the softmax denominator in row 64.
  flush:   per (batch, q-tile): DVE reciprocal reads the two denominator rows
           straight out of the PSUM accumulators; a single f32r outer-product
           matmul (selector [2,128] x recips [2,512]) broadcasts both heads'
           reciprocals across the 128 partitions; two DVE multiplies read the
           accumulators directly from PSUM and write normalized bf16 outT.
  phase 3: partial out = outT.T @ W_out_rows -> bf16 [tokens, 1024].

DMA: weights (contiguous per-partition layout) + q chunks ride the scalar
HWDGE queue, kv chunks + outputs the sync HWDGE queue, so the startup
critical path (wk -> kv0 -> kproj0 -> qproj0 -> first scores) is ~7us
shorter than a single-queue layout.  Emission is a hand-tuned interleave:
the dependency-driven Tile scheduler always has dep-free PE work (projection
chunks, out-projection quarters) inside the ACT(exp)-bound attention stream,
with out-projection pieces reserved as PE filler for the tail flushes.
"""

import os
import sys

for _p in ("/opt/trn_rl_repo", "/root/.axon_site/_ro/trn_rl_repo"):
    if os.path.isdir(_p) and _p not in sys.path:
        sys.path.append(_p)

import ml_dtypes
import numpy as np

import concourse.bass as bass  # noqa: F401
import concourse.tile as tile
from concourse import bacc, mybir
from concourse.bass_utils import run_bass_kernel_spmd
from concourse.masks import make_identity

B, SQ, SKV, DIM = 2, 2048, 2048, 1024
HEADS, DH = 16, 64
N_CORES = 8
HPC = HEADS // N_CORES  # heads per core = 2
HD = HPC * DH  # 128 head-dim rows per core
TOK = B * SQ  # 4096
KO = DIM // 128  # 8 contraction chunks of 128
SCALE = DH**-0.5

BF16 = mybir.dt.bfloat16
F32 = mybir.dt.float32

PCHUNK = 512  # token chunk in projections (contiguous per-chunk dram layout)
QTILE = 512  # q tile in attention
KTILE = 128  # k tile (scores psum partition dim)
NKT = SKV // KTILE  # 16
NQT = SQ // QTILE  # 4
NCH = SQ // PCHUNK  # 4 chunks per batch

BF = ml_dtypes.bfloat16
Exp = mybir.ActivationFunctionType.Exp

LOOKAHEAD = 6


def build():
    nc = bacc.Bacc(
        "TRN2", target_bir_lowering=False, debug=False, num_devices=N_CORES
    )

    xqt_d = nc.dram_tensor(
        "xqt", [B * NCH, 128, KO, PCHUNK], BF16, kind="ExternalInput"
    )
    xkvt_d = nc.dram_tensor(
        "xkvt", [B * NCH, 128, KO, PCHUNK], BF16, kind="ExternalInput"
    )
    # weights pre-rearranged on host to partition-major contiguous layout
    wq_d = nc.dram_tensor("wq", [128, KO, HD], BF16, kind="ExternalInput")
    wk_d = nc.dram_tensor("wk", [128, KO, HD], BF16, kind="ExternalInput")
    wv_d = nc.dram_tensor("wv", [128, KO, HD], BF16, kind="ExternalInput")
    wout_d = nc.dram_tensor("wout", [HD, DIM], BF16, kind="ExternalInput")
    out_d = nc.dram_tensor("out", [TOK, DIM], BF16, kind="ExternalOutput")

    xqt = xqt_d.ap()
    xkvt = xkvt_d.ap()
    F32R = mybir.dt.float32r

    with tile.TileContext(nc) as tc:
        with (
            tc.tile_pool(name="persist", bufs=1) as persist,
            tc.tile_pool(name="xin", bufs=8) as xin,
            tc.tile_pool(name="exps", bufs=10) as exps,
            tc.tile_pool(name="ost", bufs=4) as ost,
            tc.tile_pool(name="nrm", bufs=4) as nrm,
            tc.tile_pool(name="spsum", bufs=2, space="PSUM") as spsum,
            tc.tile_pool(name="accp", bufs=2, space="PSUM") as accp,
            tc.tile_pool(name="miscp", bufs=2, space="PSUM") as miscp,
        ):
            # --- weights on the scalar HWDGE queue: wk, wq first (startup
            # critical path), wv/wout behind the first q chunk ---
            wk_sb = persist.tile([128, KO, HD], BF16, tag="wk")
            nc.scalar.dma_start(wk_sb[:], wk_d.ap())
            wq_sb = persist.tile([128, KO, HD], BF16, tag="wq")
            nc.scalar.dma_start(wq_sb[:], wq_d.ap())

            xts = {}  # (b, c) -> loaded x_kv chunk tile

            def kv_load(b, c):
                t = xin.tile([128, KO, PCHUNK], BF16, tag="x")
                nc.sync.dma_start(t[:], xkvt[b * NCH + c])
                xts[(b, c)] = t

            def q_load(b, c):
                t = xin.tile([128, KO, PCHUNK], BF16, tag="x")
                nc.scalar.dma_start(t[:], xqt[b * NCH + c])
                xts[("q", b, c)] = t

            kv_load(0, 0)
            q_load(0, 0)
            kv_load(0, 1)

            wv_sb = persist.tile([128, KO, HD], BF16, tag="wv")
            nc.scalar.dma_start(wv_sb[:], wv_d.ap())
            wout_sb = persist.tile([HD, DIM], BF16, tag="wout")
            nc.scalar.dma_start(wout_sb[:], wout_d.ap())

            # --- constants ---
            ident = persist.tile([128, DH], BF16, tag="ident")
            make_identity(nc, ident[0:DH, :])
            make_identity(nc, ident[DH : 2 * DH, :])
            # head-selector for the reciprocal broadcast: partition h*64
            # covers output partitions [h*64, (h+1)*64); rows 1-63 stay zero
            st2 = persist.tile([DH + 1, 128], F32, tag="st2")
            nc.vector.memset(st2[:], 0.0)
            nc.vector.memset(st2[0:1, 0:DH], 1.0)
            nc.vector.memset(st2[DH : DH + 1, DH : 2 * DH], 1.0)
            # reciprocal staging at partitions {0, 64}; rows 1-63 are zeroed
            # once and never written again, keeping the f32r contraction
            # exact.  f32r dtype so the DVE reciprocal rounds on write (the
            # f32r matmul consumer requires a rounded producer).
            rcps = []
            for i in range(2):
                r = persist.tile([DH + 1, QTILE], F32R, tag=f"rcp{i}")
                nc.vector.memset(r[:].bitcast(F32), 0.0)
                rcps.append(r)
            # prefetch the exp table set during the initial DMAs
            dummy = persist.tile([1, 8], F32, tag="dummy")
            nc.vector.memset(dummy[:], 0.0)
            nc.scalar.activation(dummy[:], dummy[:], Exp)

            qt_sb, kt_sb, vt_sb, vnat, outT = {}, {}, {}, {}, {}
            for b in range(B):
                qt_sb[b] = persist.tile([HD, SQ], BF16, tag=f"qt{b}", name=f"qt{b}")
                kt_sb[b] = persist.tile([HD, SKV], BF16, tag=f"kt{b}", name=f"kt{b}")
                vt_sb[b] = persist.tile([HD, SKV], BF16, tag=f"vt{b}", name=f"vt{b}")
                vnat[b] = persist.tile(
                    [128, HPC, NKT, DH + 1], BF16, tag=f"vn{b}", name=f"vn{b}"
                )
                outT[b] = persist.tile([HD, SQ], BF16, tag=f"ot{b}", name=f"ot{b}")
                nc.vector.memset(vnat[b][:, :, :, DH], 1.0)

            def _proj(dst, w_sb, xt, c):
                ps = miscp.tile([128, PCHUNK], F32, tag="m", name="projp")
                for ko in range(KO):
                    nc.tensor.matmul(
                        ps[:],
                        w_sb[:, ko, :],
                        xt[:, ko, :],
                        start=(ko == 0),
                        stop=(ko == KO - 1),
                    )
                nc.vector.tensor_copy(
                    dst[:, c * PCHUNK : (c + 1) * PCHUNK], ps[:]
                )

            def kproj(b, c):
                _proj(kt_sb[b], wk_sb, xts[(b, c)], c)

            def qproj(b, c):
                _proj(qt_sb[b], wq_sb, xts.pop(("q", b, c)), c)

            def vproj(b, c):
                """V projection for chunk c + PE-transpose into natural
                layout (k-tiles 4c..4c+3); frees the x chunk tile."""
                _proj(vt_sb[b], wv_sb, xts.pop((b, c)), c)
                for h in range(HPC):
                    tp = miscp.tile([128, 4, DH], BF16, tag="m", name="vtp")
                    for i in range(4):
                        j = c * 4 + i
                        nc.tensor.transpose(
                            tp[:, i, :],
                            vt_sb[b][
                                h * DH : (h + 1) * DH,
                                j * KTILE : (j + 1) * KTILE,
                            ],
                            ident[h * DH : (h + 1) * DH, :],
                        )
                    nc.vector.tensor_copy(
                        vnat[b][:, h, c * 4 : (c + 1) * 4, 0:DH], tp[:]
                    )

            # --- attention ---
            acc_store = {0: {}, 1: {}}

            def attention(b, pre, post):
                """Flat software-pipelined attention over all (qt, j) steps.

                Scores for step t+LOOKAHEAD are emitted before PV of step t,
                so the PE always has score matmuls queued ahead of the
                exp/PV chain.  pre[t] hooks fire before scores(t); post[s]
                hooks fire right after step s's PV matmuls.
                """
                NT = NQT * NKT
                sps = {}
                accs = acc_store[b]

                def emit_scores(t):
                    qt, j = divmod(t, NKT)
                    q_sl = slice(qt * QTILE, (qt + 1) * QTILE)
                    k_sl = slice(j * KTILE, (j + 1) * KTILE)
                    sp = spsum.tile([128, HPC, QTILE], F32, tag="s", name="sp")
                    sps[t] = sp
                    for h in range(HPC):
                        h_sl = slice(h * DH, (h + 1) * DH)
                        nc.tensor.matmul(
                            sp[:, h, :],
                            kt_sb[b][h_sl, k_sl],
                            qt_sb[b][h_sl, q_sl],
                            start=True,
                            stop=True,
                        )

                def emit_tail(t):
                    qt, j = divmod(t, NKT)
                    sp = sps.pop(t)
                    ex = exps.tile([128, HPC, QTILE], BF16, tag="e", name="ex")
                    nc.scalar.activation(ex[:], sp[:], Exp, scale=SCALE)
                    if j == 0:
                        accs[qt] = [
                            accp.tile([128, QTILE], F32, tag="acc", name="acc")
                            for _ in range(HPC)
                        ]
                    for h in range(HPC):
                        nc.tensor.matmul(
                            accs[qt][h][0 : DH + 1, :],
                            vnat[b][:, h, j, :],
                            ex[:, h, :],
                            start=(j == 0),
                            stop=(j == NKT - 1),
                        )

                for t in range(NT + LOOKAHEAD):
                    for fn in pre.get(t, ()):
                        fn()
                    if t < NT:
                        emit_scores(t)
                    if t >= LOOKAHEAD:
                        emit_tail(t - LOOKAHEAD)
                        for fn in post.get(t - LOOKAHEAD, ()):
                            fn()

            _flno = [0]

            def fl_recips(b, qt):
                """DVE reciprocals of the two denominator rows (read straight
                from the PSUM accumulators), then stage the unnormalized
                accumulators into SBUF, freeing the PSUM banks."""
                rcp = rcps[_flno[0] % 2]
                _flno[0] += 1
                ucp = nrm.tile([DH, HPC, QTILE], F32, tag="u", name="ucp")
                accs = acc_store[b].pop(qt)
                with nc.allow_low_precision(
                    reason="f32r reciprocal feeding f32r broadcast matmul"
                ):
                    for h in range(HPC):
                        nc.vector.reciprocal(
                            rcp[h * DH : h * DH + 1, :],
                            accs[h][DH : DH + 1, :],
                        )
                for h in range(HPC):
                    nc.vector.tensor_copy(ucp[:, h, :], accs[h][0:DH, :])
                return rcp, ucp

            def fl_norm(b, qt, rcp, ucp):
                """Broadcast both heads' reciprocals across partitions with
                one f32r outer product, then normalize into bf16 outT."""
                q_sl = slice(qt * QTILE, (qt + 1) * QTILE)
                bcp = miscp.tile([128, QTILE], F32, tag="m", name="bcp")
                nc.tensor.matmul(
                    bcp[:],
                    st2[:].bitcast(F32R),
                    rcp[:],
                    start=True,
                    stop=True,
                )
                for h in range(HPC):
                    h_sl = slice(h * DH, (h + 1) * DH)
                    nc.vector.tensor_mul(
                        outT[b][h_sl, q_sl], ucp[:, h, :], bcp[h_sl, :]
                    )

            def flush(b, qt, filler=None):
                """Reciprocals + PSUM drain first (DVE), then a PE filler
                piece to cover their latency, then broadcast+normalize."""
                rcp, ucp = fl_recips(b, qt)
                if filler is not None:
                    filler()
                fl_norm(b, qt, rcp, ucp)

            def op(b, tt, split_copy=False):
                """Out-projection for one 128-token chunk + output DMA."""
                t_sl = slice(tt * 128, (tt + 1) * 128)
                ob = ost.tile([128, 2, 512], BF16, tag="o")
                for nt in range(DIM // 512):
                    ps = miscp.tile([128, 512], F32, tag="m", name="projo")
                    nc.tensor.matmul(
                        ps[:],
                        outT[b][:, t_sl],
                        wout_sb[:, nt * 512 : (nt + 1) * 512],
                        start=True,
                        stop=True,
                    )
                    if split_copy and nt % 2 == 0:
                        nc.scalar.copy(ob[:, nt, :], ps[:])
                    else:
                        nc.vector.tensor_copy(ob[:, nt, :], ps[:])
                nc.sync.dma_start(
                    out_d.ap()[
                        b * SQ + tt * 128 : b * SQ + (tt + 1) * 128, :
                    ].rearrange("t (n c) -> t n c", n=2),
                    ob[:],
                )

            # --- startup: first projections, then attention begins ---
            kproj(0, 0)
            qproj(0, 0)

            L = lambda fn, *a, **k: (lambda: fn(*a, **k))

            pre0 = {
                2: [L(kv_load, 0, 2)],
                4: [L(kproj, 0, 1), L(kv_load, 0, 3)],
                6: [L(vproj, 0, 0), L(q_load, 0, 1)],
                8: [L(kproj, 0, 2)],
                12: [L(kproj, 0, 3)],
            }
            post0 = {
                3: [L(vproj, 0, 1)],
                6: [L(q_load, 0, 2)],
                7: [L(vproj, 0, 2)],
                8: [L(kv_load, 1, 0)],
                9: [L(qproj, 0, 1)],
                11: [L(vproj, 0, 3)],
                14: [L(q_load, 0, 3)],
                15: [L(flush, 0, 0, L(kproj, 1, 0))],
                16: [L(op, 0, 0)],
                17: [L(op, 0, 1)],
                18: [L(q_load, 1, 0)],
                19: [L(qproj, 0, 2)],
                21: [L(op, 0, 2)],
                23: [L(op, 0, 3)],
                25: [L(kv_load, 1, 1)],
                26: [L(vproj, 1, 0)],
                29: [L(qproj, 0, 3)],
                31: [L(flush, 0, 1, L(kproj, 1, 1))],
                32: [L(op, 0, 4)],
                33: [L(kv_load, 1, 2), L(q_load, 1, 1)],
                34: [L(op, 0, 5)],
                35: [L(vproj, 1, 1)],
                37: [L(qproj, 1, 0)],
                39: [L(op, 0, 6)],
                40: [L(kv_load, 1, 3), L(q_load, 1, 2)],
                41: [L(op, 0, 7)],
                43: [L(kproj, 1, 2)],
                45: [L(vproj, 1, 2)],
                47: [L(flush, 0, 2, L(kproj, 1, 3))],
                48: [L(op, 0, 8)],
                49: [L(vproj, 1, 3)],
                50: [L(q_load, 1, 3)],
                51: [L(qproj, 1, 1)],
                53: [L(op, 0, 9)],
                55: [L(op, 0, 10)],
                57: [L(op, 0, 11)],
                59: [L(qproj, 1, 2)],
                63: [L(flush, 0, 3, L(qproj, 1, 3))],
            }
            attention(0, pre0, post0)

            post1 = {
                0: [L(op, 0, 12)],
                2: [L(op, 0, 13)],
                15: [L(flush, 1, 0, L(op, 0, 14))],
                16: [L(op, 1, 0)],
                18: [L(op, 1, 1)],
                31: [L(flush, 1, 1, L(op, 0, 15))],
                32: [L(op, 1, 4)],
                34: [L(op, 1, 5)],
                47: [L(flush, 1, 2, L(op, 1, 2))],
                48: [L(op, 1, 8)],
                50: [L(op, 1, 9)],
                53: [L(op, 1, 3)],
                55: [L(op, 1, 6)],
                57: [L(op, 1, 7)],
                59: [L(op, 1, 10)],
                63: [L(flush, 1, 3, L(op, 1, 11))],
            }
            attention(1, {}, post1)
            op(1, 12, split_copy=True)
            op(1, 13, split_copy=True)
            op(1, 14, split_copy=True)
            op(1, 15, split_copy=True)

    nc.compile()
    return nc


def make_in_maps(x_q, x_kv, W_qkv, W_out):
    x_q = np.asarray(x_q, dtype=np.float32)
    x_kv = np.asarray(x_kv, dtype=np.float32)
    W_qkv = np.asarray(W_qkv, dtype=np.float32)
    W_out = np.asarray(W_out, dtype=np.float32)

    def chunk_tile(x):
        # [TOK, DIM] -> [n_chunks, 128, KO, PCHUNK] with D = ko*128 + p
        xt = x.reshape(TOK, DIM).T.reshape(KO, 128, TOK // PCHUNK, PCHUNK)
        return np.ascontiguousarray(xt.transpose(2, 1, 0, 3)).astype(BF)

    def w_tile(w):
        # [DIM, HD] -> [128, KO, HD] partition-major contiguous
        return np.ascontiguousarray(
            w.reshape(KO, 128, HD).transpose(1, 0, 2)
        ).astype(BF)

    xqt = chunk_tile(x_q)
    xkvt = chunk_tile(x_kv)

    in_maps = []
    for c in range(N_CORES):
        cs = slice(c * HD, (c + 1) * HD)
        in_maps.append(
            {
                "xqt": xqt,
                "xkvt": xkvt,
                "wq": w_tile(W_qkv[:, cs]),
                "wk": w_tile(W_qkv[:, 1024:][:, cs]),
                "wv": w_tile(W_qkv[:, 2048:][:, cs]),
                "wout": np.ascontiguousarray(W_out[cs, :]).astype(BF),
            }
        )
    return in_maps


def combine(partials, b_out):
    """Sum the 8 per-core partial projections and add the bias."""
    acc = np.zeros((TOK, DIM), dtype=np.float32)
    for p in partials:
        acc += np.asarray(p, dtype=np.float32)
    acc += np.asarray(b_out, dtype=np.float32)
    return acc.reshape(B, SQ, DIM)


_STATE = {}


def _get_nc():
    if "nc" not in _STATE:
        _STATE["nc"] = build()
    return _STATE["nc"]


def run(x_q, x_kv, W_qkv, W_out, b_out, trace=False):
    nc = _get_nc()
    in_maps = make_in_maps(x_q, x_kv, W_qkv, W_out)
    res = run_bass_kernel_spmd(nc, in_maps, list(range(N_CORES)), trace=trace)
    out = combine([r["out"] for r in res.results], b_out)
    return out, res


def kernel(x_q, x_kv, W_qkv, W_out, b_out):
    out, _ = run(x_q, x_kv, W_qkv, W_out, b_out, trace=False)
    return out


# revision 26
# speedup vs baseline: 1.1863x; 1.1863x over previous
"""Multi-head attention (b=2, sq=skv=2048, dim=1024, 16 heads x 64) on 8 TRN2
NeuronCores.

Sharding: 2 heads per core (head-parallel), with the matching tensor-parallel
column slice of W_qkv and row slice of W_out.  Each core computes a partial
output projection over its 128 head-dims; the all-reduce of the 8 partials
(+ bias) happens on the host during unshard.

Per-core kernel (bf16 compute, fp32 PSUM accumulation):
  phase 1: qT/kT/vT = W.T @ x.T ([128 = 2 heads x 64 dims, tokens]); v is
           additionally PE-transposed to natural [token, dim] layout with a
           ones column appended (denominator trick).
  phase 2: per (batch, q-tile, k-tile): scoresT for both heads ([k-tokens, q])
           in one 2-bank PSUM group; one exp ACTIVATE over the group (scale
           1/8 fused, no max subtraction -- scores range +-10); PV matmuls
           accumulate [v | 1].T @ expT over the 16 k-tiles giving unnormalized
           outT plus the softmax denominator in row 64.
  flush:   per (batch, q-tile): DVE reciprocal reads the two denominator rows
           straight out of the PSUM accumulators; a single f32r outer-product
           matmul (selector [2,128] x recips [2,512]) broadcasts both heads'
           reciprocals across the 128 partitions; two DVE multiplies read the
           accumulators directly from PSUM and write normalized bf16 outT.
  phase 3: partial out = outT.T @ W_out_rows -> bf16 [tokens, 1024].

DMA: weights (contiguous per-partition layout) + q chunks ride the scalar
HWDGE queue, kv chunks + outputs the sync HWDGE queue, so the startup
critical path (wk -> kv0 -> kproj0 -> qproj0 -> first scores) is ~7us
shorter than a single-queue layout.  Emission is a hand-tuned interleave:
the dependency-driven Tile scheduler always has dep-free PE work (projection
chunks, out-projection quarters) inside the ACT(exp)-bound attention stream,
with out-projection pieces reserved as PE filler for the tail flushes.
"""

import os
import sys

for _p in ("/opt/trn_rl_repo", "/root/.axon_site/_ro/trn_rl_repo"):
    if os.path.isdir(_p) and _p not in sys.path:
        sys.path.append(_p)

import ml_dtypes
import numpy as np

import concourse.bass as bass  # noqa: F401
import concourse.tile as tile
from concourse import bacc, mybir
from concourse.bass_utils import run_bass_kernel_spmd
from concourse.masks import make_identity

B, SQ, SKV, DIM = 2, 2048, 2048, 1024
HEADS, DH = 16, 64
N_CORES = 8
HPC = HEADS // N_CORES  # heads per core = 2
HD = HPC * DH  # 128 head-dim rows per core
TOK = B * SQ  # 4096
KO = DIM // 128  # 8 contraction chunks of 128
SCALE = DH**-0.5

BF16 = mybir.dt.bfloat16
F32 = mybir.dt.float32

PCHUNK = 512  # token chunk in projections (contiguous per-chunk dram layout)
QTILE = 512  # q tile in attention
KTILE = 128  # k tile (scores psum partition dim)
NKT = SKV // KTILE  # 16
NQT = SQ // QTILE  # 4
NCH = SQ // PCHUNK  # 4 chunks per batch

BF = ml_dtypes.bfloat16
Exp = mybir.ActivationFunctionType.Exp

LOOKAHEAD = 6


def build():
    nc = bacc.Bacc(
        "TRN2", target_bir_lowering=False, debug=False, num_devices=N_CORES
    )

    xqt_d = nc.dram_tensor(
        "xqt", [B * NCH, 128, KO, PCHUNK], BF16, kind="ExternalInput"
    )
    xkvt_d = nc.dram_tensor(
        "xkvt", [B * NCH, 128, KO, PCHUNK], BF16, kind="ExternalInput"
    )
    # weights pre-rearranged on host to partition-major contiguous layout
    wq_d = nc.dram_tensor("wq", [128, KO, HD], BF16, kind="ExternalInput")
    wk_d = nc.dram_tensor("wk", [128, KO, HD], BF16, kind="ExternalInput")
    wv_d = nc.dram_tensor("wv", [128, KO, HD], BF16, kind="ExternalInput")
    wout_d = nc.dram_tensor("wout", [HD, DIM], BF16, kind="ExternalInput")
    out_d = nc.dram_tensor("out", [TOK, DIM], BF16, kind="ExternalOutput")

    xqt = xqt_d.ap()
    xkvt = xkvt_d.ap()
    F32R = mybir.dt.float32r

    with tile.TileContext(nc) as tc:
        with (
            tc.tile_pool(name="persist", bufs=1) as persist,
            tc.tile_pool(name="xin", bufs=8) as xin,
            tc.tile_pool(name="exps", bufs=10) as exps,
            tc.tile_pool(name="ost", bufs=4) as ost,
            tc.tile_pool(name="nrm", bufs=3) as nrm,
            tc.tile_pool(name="pkp", bufs=2) as pkp,
            tc.tile_pool(name="spsum", bufs=2, space="PSUM") as spsum,
            tc.tile_pool(name="accp", bufs=2, space="PSUM") as accp,
            tc.tile_pool(name="miscp", bufs=2, space="PSUM") as miscp,
            tc.tile_pool(name="drp", bufs=2, space="DRAM") as drp,
        ):
            # --- weights on the scalar HWDGE queue: wk, wq first (startup
            # critical path), wv/wout behind the first q chunk ---
            wk_sb = persist.tile([128, KO, HD], BF16, tag="wk")
            nc.scalar.dma_start(wk_sb[:], wk_d.ap())
            wq_sb = persist.tile([128, KO, HD], BF16, tag="wq")
            nc.scalar.dma_start(wq_sb[:], wq_d.ap())

            xts = {}  # (b, c) -> loaded x_kv chunk tile

            def kv_load(b, c):
                t = xin.tile([128, KO, PCHUNK], BF16, tag="x")
                nc.sync.dma_start(t[:], xkvt[b * NCH + c])
                xts[(b, c)] = t

            def q_load(b, c):
                t = xin.tile([128, KO, PCHUNK], BF16, tag="x")
                nc.scalar.dma_start(t[:], xqt[b * NCH + c])
                xts[("q", b, c)] = t

            kv_load(0, 0)
            q_load(0, 0)
            kv_load(0, 1)

            wv_sb = persist.tile([128, KO, HD], BF16, tag="wv")
            nc.scalar.dma_start(wv_sb[:], wv_d.ap())
            wout_sb = persist.tile([HD, DIM], BF16, tag="wout")
            nc.scalar.dma_start(wout_sb[:], wout_d.ap())

            # --- constants ---
            ident = persist.tile([128, DH], BF16, tag="ident")
            make_identity(nc, ident[0:DH, :])
            make_identity(nc, ident[DH : 2 * DH, :])
            # head-selector for the reciprocal broadcast: partition h*64
            # covers output partitions [h*64, (h+1)*64); rows 1-63 stay zero
            st2 = persist.tile([DH + 1, 128], F32, tag="st2")
            nc.vector.memset(st2[:], 0.0)
            nc.vector.memset(st2[0:1, 0:DH], 1.0)
            nc.vector.memset(st2[DH : DH + 1, DH : 2 * DH], 1.0)
            # reciprocal staging at partitions {0, 64}; rows 1-63 are zeroed
            # once and never written again, keeping the f32r contraction exact
            rcps = []
            for i in range(2):
                r = persist.tile([DH + 1, QTILE], F32, tag=f"rcp{i}")
                nc.vector.memset(r[:], 0.0)
                rcps.append(r)
            # prefetch the exp table set during the initial DMAs
            dummy = persist.tile([1, 8], F32, tag="dummy")
            nc.vector.memset(dummy[:], 0.0)
            nc.scalar.activation(dummy[:], dummy[:], Exp)

            qt_sb, kt_sb, vt_sb, vnat, outT = {}, {}, {}, {}, {}
            for b in range(B):
                qt_sb[b] = persist.tile([HD, SQ], BF16, tag=f"qt{b}", name=f"qt{b}")
                kt_sb[b] = persist.tile([HD, SKV], BF16, tag=f"kt{b}", name=f"kt{b}")
                vt_sb[b] = persist.tile([HD, SKV], BF16, tag=f"vt{b}", name=f"vt{b}")
                vnat[b] = persist.tile(
                    [128, HPC, NKT, DH + 1], BF16, tag=f"vn{b}", name=f"vn{b}"
                )
                outT[b] = persist.tile([HD, SQ], BF16, tag=f"ot{b}", name=f"ot{b}")
                nc.vector.memset(vnat[b][:, :, :, DH], 1.0)

            def _proj(dst, w_sb, xt, c):
                ps = miscp.tile([128, PCHUNK], F32, tag="m", name="projp")
                for ko in range(KO):
                    nc.tensor.matmul(
                        ps[:],
                        w_sb[:, ko, :],
                        xt[:, ko, :],
                        start=(ko == 0),
                        stop=(ko == KO - 1),
                    )
                nc.vector.tensor_copy(
                    dst[:, c * PCHUNK : (c + 1) * PCHUNK], ps[:]
                )

            def kproj(b, c):
                _proj(kt_sb[b], wk_sb, xts[(b, c)], c)

            def qproj(b, c):
                _proj(qt_sb[b], wq_sb, xts.pop(("q", b, c)), c)

            def vproj(b, c):
                """V projection for chunk c + PE-transpose into natural
                layout (k-tiles 4c..4c+3); frees the x chunk tile."""
                _proj(vt_sb[b], wv_sb, xts.pop((b, c)), c)
                for h in range(HPC):
                    tp = miscp.tile([128, 4, DH], BF16, tag="m", name="vtp")
                    for i in range(4):
                        j = c * 4 + i
                        nc.tensor.transpose(
                            tp[:, i, :],
                            vt_sb[b][
                                h * DH : (h + 1) * DH,
                                j * KTILE : (j + 1) * KTILE,
                            ],
                            ident[h * DH : (h + 1) * DH, :],
                        )
                    nc.vector.tensor_copy(
                        vnat[b][:, h, c * 4 : (c + 1) * 4, 0:DH], tp[:]
                    )

            # --- attention ---
            acc_store = {0: {}, 1: {}}
            ucps = {}

            def attention(b, pre, post):
                """Flat software-pipelined attention over all (qt, j) steps.

                Scores for step t+LOOKAHEAD are emitted before PV of step t,
                so the PE always has score matmuls queued ahead of the
                exp/PV chain.  pre[t] hooks fire before scores(t); post[s]
                hooks fire right after step s's PV matmuls.
                """
                NT = NQT * NKT
                sps = {}
                accs = acc_store[b]

                def emit_scores(t):
                    qt, j = divmod(t, NKT)
                    q_sl = slice(qt * QTILE, (qt + 1) * QTILE)
                    k_sl = slice(j * KTILE, (j + 1) * KTILE)
                    sp = spsum.tile([128, HPC, QTILE], F32, tag="s", name="sp")
                    sps[t] = sp
                    for h in range(HPC):
                        h_sl = slice(h * DH, (h + 1) * DH)
                        nc.tensor.matmul(
                            sp[:, h, :],
                            kt_sb[b][h_sl, k_sl],
                            qt_sb[b][h_sl, q_sl],
                            start=True,
                            stop=True,
                        )

                def emit_tail(t):
                    qt, j = divmod(t, NKT)
                    sp = sps.pop(t)
                    ex = exps.tile([128, HPC, QTILE], BF16, tag="e", name="ex")
                    nc.scalar.activation(ex[:], sp[:], Exp, scale=SCALE)
                    if j == 0:
                        accs[qt] = [
                            accp.tile([128, QTILE], F32, tag="acc", name="acc")
                            for _ in range(HPC)
                        ]
                    for h in range(HPC):
                        nc.tensor.matmul(
                            accs[qt][h][0 : DH + 1, :],
                            vnat[b][:, h, j, :],
                            ex[:, h, :],
                            start=(j == 0),
                            stop=(j == NKT - 1),
                        )
                    if j == NKT - 1:
                        # drain the PSUM accumulators (incl. denominator row)
                        # to SBUF immediately; normalization is deferred
                        ucp = nrm.tile(
                            [DH + 1, HPC, QTILE], F32, tag="u", name="ucp"
                        )
                        acc2 = accs.pop(qt)
                        for h in range(HPC):
                            nc.vector.tensor_copy(
                                ucp[:, h, :], acc2[h][0 : DH + 1, :]
                            )
                        ucps[(b, qt)] = ucp

                for t in range(NT + LOOKAHEAD):
                    for fn in pre.get(t, ()):
                        fn()
                    if t < NT:
                        emit_scores(t)
                    if t >= LOOKAHEAD:
                        emit_tail(t - LOOKAHEAD)
                        for fn in post.get(t - LOOKAHEAD, ()):
                            fn()

            _flno = [0]

            def flush(b, qt):
                """Deferred normalization for one (batch, q-tile).

                The two denominator rows are bounced through DRAM to repack
                [1, 2, 512] -> [128, 8] so the DVE reciprocal runs wide (a
                [1, 512] reciprocal costs ~3.4us on the DVE; [128, 8] costs
                ~0.15us), bounced back to row form at partitions {0, 64},
                broadcast across partitions with a single f32r outer product
                against the head-selector, and multiplied into bf16 outT.
                Runs ~8 steps after the q-tile's last PV, so the DMA bounce
                latency sits off the critical path.
                """
                ucp = ucps.pop((b, qt))
                q_sl = slice(qt * QTILE, (qt + 1) * QTILE)
                rcp = rcps[_flno[0] % 2]
                _flno[0] += 1
                d1 = drp.tile([1, HPC, QTILE], F32, tag="d1", name="d1")
                nc.gpsimd.dma_start(d1[:], ucp[DH : DH + 1, :, :])
                dpk = pkp.tile([128, HPC * QTILE // 128], F32, tag="dp")
                nc.gpsimd.dma_start(
                    dpk[:],
                    d1[:]
                    .rearrange("a b c -> (a b c)")
                    .rearrange("(p f) -> p f", p=128),
                )
                rpk = pkp.tile([128, HPC * QTILE // 128], F32, tag="rp")
                nc.vector.reciprocal(rpk[:], dpk[:])
                d2 = drp.tile([128, HPC * QTILE // 128], F32, tag="d2")
                nc.gpsimd.dma_start(d2[:], rpk[:])
                d2f = d2[:].rearrange("p f -> (p f)")
                for h in range(HPC):
                    nc.gpsimd.dma_start(
                        rcp[h * DH : h * DH + 1, :],
                        d2f[h * QTILE : (h + 1) * QTILE].rearrange(
                            "(a c) -> a c", a=1
                        ),
                    )
                bcp = miscp.tile([128, QTILE], F32, tag="m", name="bcp")
                nc.tensor.matmul(
                    bcp[:],
                    st2[:].bitcast(F32R),
                    rcp[:].bitcast(F32R),
                    start=True,
                    stop=True,
                )
                for h in range(HPC):
                    h_sl = slice(h * DH, (h + 1) * DH)
                    nc.vector.tensor_mul(
                        outT[b][h_sl, q_sl], ucp[0:DH, h, :], bcp[h_sl, :]
                    )

            def op(b, tt, split_copy=False):
                """Out-projection for one 128-token chunk + output DMA."""
                t_sl = slice(tt * 128, (tt + 1) * 128)
                ob = ost.tile([128, 2, 512], BF16, tag="o")
                for nt in range(DIM // 512):
                    ps = miscp.tile([128, 512], F32, tag="m", name="projo")
                    nc.tensor.matmul(
                        ps[:],
                        outT[b][:, t_sl],
                        wout_sb[:, nt * 512 : (nt + 1) * 512],
                        start=True,
                        stop=True,
                    )
                    if split_copy and nt % 2 == 0:
                        nc.scalar.copy(ob[:, nt, :], ps[:])
                    else:
                        nc.vector.tensor_copy(ob[:, nt, :], ps[:])
                nc.sync.dma_start(
                    out_d.ap()[
                        b * SQ + tt * 128 : b * SQ + (tt + 1) * 128, :
                    ].rearrange("t (n c) -> t n c", n=2),
                    ob[:],
                )

            # --- startup: first projections, then attention begins ---
            kproj(0, 0)
            qproj(0, 0)

            L = lambda fn, *a, **k: (lambda: fn(*a, **k))

            pre0 = {
                2: [L(kv_load, 0, 2)],
                4: [L(kproj, 0, 1), L(kv_load, 0, 3)],
                6: [L(vproj, 0, 0), L(q_load, 0, 1)],
                8: [L(kproj, 0, 2)],
                12: [L(kproj, 0, 3)],
            }
            post0 = {
                3: [L(vproj, 0, 1)],
                6: [L(q_load, 0, 2)],
                7: [L(vproj, 0, 2)],
                8: [L(kv_load, 1, 0)],
                9: [L(qproj, 0, 1)],
                11: [L(vproj, 0, 3)],
                14: [L(q_load, 0, 3)],
                15: [L(kproj, 1, 0)],
                17: [L(qproj, 0, 2)],
                19: [L(kv_load, 1, 1), L(q_load, 1, 0)],
                23: [L(flush, 0, 0)],
                24: [L(op, 0, 0)],
                25: [L(kproj, 1, 1)],
                26: [L(op, 0, 1)],
                27: [L(vproj, 1, 0)],
                28: [L(op, 0, 2)],
                29: [L(qproj, 0, 3)],
                30: [L(op, 0, 3)],
                33: [L(kv_load, 1, 2), L(q_load, 1, 1)],
                35: [L(vproj, 1, 1)],
                37: [L(qproj, 1, 0)],
                39: [L(flush, 0, 1)],
                40: [L(op, 0, 4)],
                41: [L(kproj, 1, 2)],
                42: [L(op, 0, 5)],
                43: [L(kv_load, 1, 3)],
                44: [L(op, 0, 6)],
                45: [L(vproj, 1, 2)],
                46: [L(op, 0, 7)],
                49: [L(q_load, 1, 2)],
                51: [L(qproj, 1, 1)],
                53: [L(kproj, 1, 3)],
                55: [L(flush, 0, 2)],
                56: [L(op, 0, 8)],
                57: [L(vproj, 1, 3), L(q_load, 1, 3)],
                58: [L(op, 0, 9)],
                60: [L(op, 0, 10)],
                61: [L(qproj, 1, 2)],
                62: [L(op, 0, 11)],
                63: [L(qproj, 1, 3)],
            }
            attention(0, pre0, post0)

            post1 = {
                0: [L(flush, 0, 3)],
                2: [L(op, 0, 12)],
                4: [L(op, 0, 13)],
                6: [L(op, 0, 14)],
                8: [L(op, 0, 15)],
                23: [L(flush, 1, 0)],
                24: [L(op, 1, 0)],
                26: [L(op, 1, 1)],
                28: [L(op, 1, 2)],
                30: [L(op, 1, 3)],
                39: [L(flush, 1, 1)],
                40: [L(op, 1, 4)],
                42: [L(op, 1, 5)],
                44: [L(op, 1, 6)],
                46: [L(op, 1, 7)],
                55: [L(flush, 1, 2)],
                56: [L(op, 1, 8)],
                58: [L(op, 1, 9)],
                60: [L(op, 1, 10)],
                62: [L(op, 1, 11)],
                63: [L(flush, 1, 3)],
            }
            attention(1, {}, post1)
            op(1, 12, split_copy=True)
            op(1, 13, split_copy=True)
            op(1, 14, split_copy=True)
            op(1, 15, split_copy=True)

    nc.compile()
    return nc


def make_in_maps(x_q, x_kv, W_qkv, W_out):
    x_q = np.asarray(x_q, dtype=np.float32)
    x_kv = np.asarray(x_kv, dtype=np.float32)
    W_qkv = np.asarray(W_qkv, dtype=np.float32)
    W_out = np.asarray(W_out, dtype=np.float32)

    def chunk_tile(x):
        # [TOK, DIM] -> [n_chunks, 128, KO, PCHUNK] with D = ko*128 + p
        xt = x.reshape(TOK, DIM).T.reshape(KO, 128, TOK // PCHUNK, PCHUNK)
        return np.ascontiguousarray(xt.transpose(2, 1, 0, 3)).astype(BF)

    def w_tile(w):
        # [DIM, HD] -> [128, KO, HD] partition-major contiguous
        return np.ascontiguousarray(
            w.reshape(KO, 128, HD).transpose(1, 0, 2)
        ).astype(BF)

    xqt = chunk_tile(x_q)
    xkvt = chunk_tile(x_kv)

    in_maps = []
    for c in range(N_CORES):
        cs = slice(c * HD, (c + 1) * HD)
        in_maps.append(
            {
                "xqt": xqt,
                "xkvt": xkvt,
                "wq": w_tile(W_qkv[:, cs]),
                "wk": w_tile(W_qkv[:, 1024:][:, cs]),
                "wv": w_tile(W_qkv[:, 2048:][:, cs]),
                "wout": np.ascontiguousarray(W_out[cs, :]).astype(BF),
            }
        )
    return in_maps


def combine(partials, b_out):
    """Sum the 8 per-core partial projections and add the bias."""
    acc = np.zeros((TOK, DIM), dtype=np.float32)
    for p in partials:
        acc += np.asarray(p, dtype=np.float32)
    acc += np.asarray(b_out, dtype=np.float32)
    return acc.reshape(B, SQ, DIM)


_STATE = {}


def _get_nc():
    if "nc" not in _STATE:
        _STATE["nc"] = build()
    return _STATE["nc"]


def run(x_q, x_kv, W_qkv, W_out, b_out, trace=False):
    nc = _get_nc()
    in_maps = make_in_maps(x_q, x_kv, W_qkv, W_out)
    res = run_bass_kernel_spmd(nc, in_maps, list(range(N_CORES)), trace=trace)
    out = combine([r["out"] for r in res.results], b_out)
    return out, res


def kernel(x_q, x_kv, W_qkv, W_out, b_out):
    out, _ = run(x_q, x_kv, W_qkv, W_out, b_out, trace=False)
    return out
